# revision 2
# baseline (speedup 1.0000x reference)
"""ClusterAssignment (Student-t / vq codebook soft-assignment) Trainium2 kernel.

Math (ALPHA=1 => power=1):
    ns[n,k]  = max(||x_n - c_k||^2, 0) = ||x||^2 + ||c||^2 - 2 x.c   (>= ~430 here, relu moot)
    num[n,k] = 1 / (1 + ns[n,k])
    out[n,k] = num[n,k] / sum_k num[n,k]

Strategy: data-parallel over 8 NeuronCores (batch N=65536 -> 8192 rows/core,
centers replicated). On each core, for every 128-row tile of the batch we
compute 1 + ns directly in PSUM via matmul using an augmented contraction:

    PSUM[j,k] = sum_c  bT[c][:,j] . (-2*cT[c][:,k])     (4 chunks of 128 over D=512)
              + xsq[n_j] * 1  +  1 * (csq[k] + 1)       (one 2-row augmented matmul)

then epilogue: num = reciprocal(PSUM) on DVE (fast approx, 18-bit), rowsum via
ScalarE Copy+accum, inv = 1/rowsum on DVE, out = num * inv on ScalarE, DMA out.

Inputs stream as bf16 (error analysis: output rel err ~1e-4, dominated by
bf16 products; the bf16 error on xsq (~±1 of ~515) is common-mode per row and
cancels in the normalization).
"""

import sys

sys.path.insert(0, "/opt/trn_rl_repo")

from contextlib import ExitStack

import ml_dtypes
import numpy as np

import concourse.bass as bass
from concourse import bacc
import concourse.mybir as mybir
import concourse.tile as tile
from concourse.bass import ts
from concourse.bass_utils import run_bass_kernel_spmd

N, K, D = 65536, 512 * 2, 512  # K=1024
NCORES = 8
NS = N // NCORES  # 8192 rows per core
NT = NS // 128  # 64 tiles per core
NCH = D // 128  # 4 contraction chunks
BF16 = mybir.dt.bfloat16
F32 = mybir.dt.float32


def build_bass():
    nc = bacc.Bacc("TRN2", target_bir_lowering=False, debug=False)
    bt = nc.declare_dram_parameter("bt", [128, NT, NCH, 128], BF16, isOutput=False)
    augb = nc.declare_dram_parameter("augb", [2, NS], BF16, isOutput=False)
    ct = nc.declare_dram_parameter("ct", [128, NCH, K], BF16, isOutput=False)
    augc = nc.declare_dram_parameter("augc", [2, K], BF16, isOutput=False)
    out = nc.declare_dram_parameter("out", [NS, K], F32, isOutput=True)

    with tile.TileContext(nc) as tc, ExitStack() as ctx:
        singles = ctx.enter_context(tc.tile_pool(name="singles", bufs=1))
        bpool = ctx.enter_context(tc.tile_pool(name="bt", bufs=3))
        npool = ctx.enter_context(tc.tile_pool(name="num", bufs=2))
        opool = ctx.enter_context(tc.tile_pool(name="outp", bufs=3))
        spool = ctx.enter_context(tc.tile_pool(name="small", bufs=6))
        psum = ctx.enter_context(tc.tile_pool(name="psum", bufs=4, space="PSUM"))

        ct_sb = singles.tile([128, NCH, K], BF16)
        nc.sync.dma_start(out=ct_sb[:], in_=ct[:])
        augb_sb = singles.tile([2, NS], BF16)
        nc.sync.dma_start(out=augb_sb[:], in_=augb[:])
        augc_sb = singles.tile([2, K], BF16)
        nc.sync.dma_start(out=augc_sb[:], in_=augc[:])

        for t in range(NT):
            bt_t = bpool.tile([128, NCH, 128], BF16)
            nc.sync.dma_start(out=bt_t[:], in_=bt[:, t])
            num = npool.tile([128, K], F32)
            for kh in range(2):
                ps = psum.tile([128, 512], F32)
                for c in range(NCH):
                    nc.tensor.matmul(
                        ps[:],
                        lhsT=bt_t[:, c, :],
                        rhs=ct_sb[:, c, ts(kh, 512)],
                        start=(c == 0),
                        stop=False,
                    )
                nc.tensor.matmul(
                    ps[:],
                    lhsT=augb_sb[:, ts(t, 128)],
                    rhs=augc_sb[:, ts(kh, 512)],
                    start=False,
                    stop=True,
                )
                # num = 1/(1+ns); values in [~400, ~700] so the fast approx
                # (18 correct bits) is safe and accurate.
                nc.vector.reciprocal_approx_fast(out=num[:, ts(kh, 512)], in_=ps[:])
            rowsum = spool.tile([128, 1], F32)
            nc.scalar.activation(
                out=num[:],
                in_=num[:],
                func=mybir.ActivationFunctionType.Copy,
                accum_out=rowsum[:],
            )
            inv = spool.tile([128, 1], F32)
            nc.vector.reciprocal(out=inv[:], in_=rowsum[:])
            o = opool.tile([128, K], F32)
            nc.scalar.activation(
                out=o[:],
                in_=num[:],
                func=mybir.ActivationFunctionType.Copy,
                scale=inv[:],
            )
            nc.sync.dma_start(out=out[ts(t, 128), :], in_=o[:])
    nc.finalize()
    return nc


_NC_CACHE = None


def _get_nc():
    global _NC_CACHE
    if _NC_CACHE is None:
        _NC_CACHE = build_bass()
    return _NC_CACHE


def prepare_inputs(batch: np.ndarray, cluster_centers: np.ndarray):
    """Host-side shard + layout. Returns in_maps for run_bass_kernel_spmd."""
    assert batch.shape == (N, D) and cluster_centers.shape == (K, D)
    b32 = batch.astype(np.float32, copy=False)
    c32 = cluster_centers.astype(np.float32, copy=False)
    xsq = np.einsum("nd,nd->n", b32, b32)  # [N]
    csq = np.einsum("kd,kd->k", c32, c32)  # [K]

    # ct[p, c, k] = -2 * centers[k, c*128+p]
    ct = (-2.0 * c32.T).reshape(NCH, 128, K).transpose(1, 0, 2)
    ct = np.ascontiguousarray(ct, dtype=ml_dtypes.bfloat16)
    augc = np.empty((2, K), dtype=ml_dtypes.bfloat16)
    augc[0] = 1.0
    augc[1] = (csq + 1.0).astype(ml_dtypes.bfloat16)

    in_maps = []
    for i in range(NCORES):
        shard = b32[i * NS : (i + 1) * NS]
        # bt[p, t, c, j] = shard[t*128+j, c*128+p]
        bt = shard.reshape(NT, 128, NCH, 128).transpose(3, 0, 2, 1)
        bt = np.ascontiguousarray(bt, dtype=ml_dtypes.bfloat16)
        augb = np.empty((2, NS), dtype=ml_dtypes.bfloat16)
        augb[0] = xsq[i * NS : (i + 1) * NS].astype(ml_dtypes.bfloat16)
        augb[1] = 1.0
        in_maps.append({"bt": bt, "augb": augb, "ct": ct, "augc": augc})
    return in_maps


def kernel(batch: np.ndarray, cluster_centers: np.ndarray, _trace=False) -> np.ndarray:
    nc = _get_nc()
    in_maps = prepare_inputs(batch, cluster_centers)
    res = run_bass_kernel_spmd(nc, in_maps, list(range(NCORES)), trace=_trace)
    out = np.concatenate([res.results[i]["out"] for i in range(NCORES)], axis=0)
    if _trace:
        return out, res
    return out
